# revision 13
# baseline (speedup 1.0000x reference)
"""Trainium2 Bass kernel for EnhancedLegalMemoRAG.

Model: 2-layer LSTM (D=768, H=512) over [B=64, S=512] -> soft attention read
over a [512, 768] memory bank -> 2-layer classifier head.

Sharding: data-parallel over batch across 8 NeuronCores (8 rows each); all
weights replicated. Inference only, so no gradient all-reduce is needed.

Per-core plan (batch b=8, rows R = S*b ordered (t, b)):
  phase 1: Xp0[R, 2048] = x @ W_ih0.T + b0          (big matmul, DRAM scratch)
  phase 2: layer-0 recurrence; h0.T for every step kept in an SBUF slab
           [128, 4, (S+1)*8] (slot t+1 = state after step t)
  phase 3: Xp1[R, 2048] = h0_seq @ W_ih1.T + b1     (lhsT comes from the slab)
  phase 4: layer-1 recurrence (only final h kept)
  phase 5: read attention (softmax over memory) + classifier

Recurrence step: gates[8, 2048] accumulate in PSUM as
  I8.T @ Xp_t  (identity-matmul injection)  +  sum_k h.T[k] @ W_hh.T[k]
with h.T chunks as the stationary operand (cheap 8-column weight loads).
Pointwise runs on ACT/DVE in the [8, 512] layout; h is transposed back to
h.T via 4 PE transposes into one PSUM bank.
"""

import functools
import sys

sys.path.insert(0, "/opt/trn_rl_repo")

import numpy as np

import concourse.bass as bass
import concourse.mybir as mybir
import concourse.tile as tile
from concourse import bacc
from concourse.bass import ts
from concourse.bass_utils import run_bass_kernel_spmd
from concourse.masks import make_identity

F32 = mybir.dt.float32
F32R = mybir.dt.float32r
AF = mybir.ActivationFunctionType
P = 128

B, S, D, H, G, M = 64, 512, 768, 512, 2048, 512
NCORES = 8
BPC = B // NCORES  # 8 batch rows per core

LAST_RESULTS = None  # BassKernelResults of the most recent run (for test.py)


def _lstm_recurrence(nc, tc, stack, s_len, ident, ident_r, w_sb, xp_dram, hT_read, hT_write, c_sb):
    """One LSTM layer, s_len serial steps.

    w_sb:    [128, 4, 2048] resident W_hh.T
    xp_dram: [s_len*BPC, 2048] precomputed input projection (+bias)
    hT_read(t)/hT_write(t): APs [128, 4, BPC] for h.T state in/out
    c_sb:    [BPC, H] persistent cell state (pre-zeroed)
    """
    xp_pool = stack.enter_context(tc.tile_pool(name="xp", bufs=4))
    gate_ps = stack.enter_context(tc.tile_pool(name="gate_ps", bufs=4, space="PSUM"))
    tp_ps = stack.enter_context(tc.tile_pool(name="tp_ps", bufs=2, space="PSUM"))
    g_pool = stack.enter_context(tc.tile_pool(name="g", bufs=2))
    ptw_pool = stack.enter_context(tc.tile_pool(name="ptw", bufs=2))

    for t in range(s_len):
        xp_sb = xp_pool.tile([BPC, G], F32R, tag="xp")
        nc.sync.dma_start(xp_sb[:], xp_dram[ts(t, BPC), :].bitcast(F32R))
        hT_prev = hT_read(t)

        gs = {}
        # order f, i, g, o: c-chain operands arrive early, o needed late
        for n in (1, 0, 2, 3):
            ps = gate_ps.tile([BPC, 512], F32, tag="gate")
            nc.tensor.matmul(
                ps[:], ident_r[:], xp_sb[:, ts(n, 512)], start=True, stop=False,
            )
            for kk in range(4):
                nc.tensor.matmul(
                    ps[:],
                    hT_prev[:, kk, :],
                    w_sb[:, kk, ts(n, 512)],
                    start=False,
                    stop=(kk == 3),
                )
            g_sb = g_pool.tile([BPC, 512], F32, tag=f"g{n}")
            nc.scalar.activation(g_sb[:], ps[:], AF.Tanh if n == 2 else AF.Sigmoid)
            gs[n] = g_sb

        # post-gate pointwise in halves of H for a shorter serial chain
        tmp = ptw_pool.tile([BPC, 512], F32, tag="tmp")
        tc_sb = ptw_pool.tile([BPC, 512], F32, tag="tc")
        h_sb = ptw_pool.tile([BPC, 512], F32, tag="h")
        tp = tp_ps.tile([P, 4, BPC], F32, tag="tp")
        hT_out = hT_write(t)
        for hh in (slice(0, 256), slice(256, 512)):
            nc.vector.tensor_mul(c_sb[:, hh], c_sb[:, hh], gs[1][:, hh])   # c *= f
            nc.vector.tensor_mul(tmp[:, hh], gs[0][:, hh], gs[2][:, hh])   # i * g
            nc.vector.tensor_add(c_sb[:, hh], c_sb[:, hh], tmp[:, hh])     # c += i*g
            nc.scalar.activation(tc_sb[:, hh], c_sb[:, hh], AF.Tanh)
            nc.vector.tensor_mul(h_sb[:, hh], gs[3][:, hh], tc_sb[:, hh])  # o * tanh(c)
            k0 = hh.start // P
            for kk in (k0, k0 + 1):
                nc.tensor.transpose(tp[:, kk, :], h_sb[:, ts(kk, P)], ident[:])
            nc.vector.tensor_copy(
                out=hT_out[:, k0 : k0 + 2, :], in_=tp[:, k0 : k0 + 2, :]
            )


def _projection(nc, tc, stack, n_mo, ko, lhsT_src, w_sb, bias_bc, out_dram):
    """out[n_mo*128, 2048] = lhsT.T @ W + bias.

    lhsT_src(mo) -> [128, ko, 128] SBUF AP; w_sb [128, ko, 2048]; bias_bc [128, 2048].
    """
    ps_pool = stack.enter_context(tc.tile_pool(name="proj_ps", bufs=4, space="PSUM"))
    out_pool = stack.enter_context(tc.tile_pool(name="proj_out", bufs=3))
    for mo in range(n_mo):
        lh = lhsT_src(mo)
        out_sb = out_pool.tile([P, G], F32, tag="proj_out")
        for n in range(4):
            ps = ps_pool.tile([P, 512], F32, tag="proj")
            for k in range(ko):
                nc.tensor.matmul(
                    ps[:],
                    lh[:, k, :],
                    w_sb[:, k, ts(n, 512)],
                    start=(k == 0),
                    stop=(k == ko - 1),
                )
            nc.vector.tensor_add(out_sb[:, ts(n, 512)], ps[:], bias_bc[:, ts(n, 512)])
        nc.sync.dma_start(out_dram[ts(mo, P), :], out_sb[:])


def _transpose_to_slab(nc, pool_ps, slab, src, nchunks, ident):
    """src [BPC, nchunks*128] -> slab [128, nchunks, BPC] via PE transposes."""
    tp = pool_ps.tile([P, nchunks, BPC], F32, tag=f"tail_tp{nchunks}")
    for kk in range(nchunks):
        nc.tensor.transpose(tp[:, kk, :], src[:, ts(kk, P)], ident[:])
    nc.vector.tensor_copy(out=slab[:], in_=tp[:])


@functools.lru_cache(maxsize=2)
def _build(s_len=S):
    R = s_len * BPC
    nc = bacc.Bacc(None, target_bir_lowering=False)

    # --- I/O ---
    xT = nc.declare_dram_parameter("xT", [D, R], F32, isOutput=False)
    wih0T = nc.declare_dram_parameter("wih0T", [D, G], F32, isOutput=False)
    whh0T = nc.declare_dram_parameter("whh0T", [H, G], F32, isOutput=False)
    wih1T = nc.declare_dram_parameter("wih1T", [H, G], F32, isOutput=False)
    whh1T = nc.declare_dram_parameter("whh1T", [H, G], F32, isOutput=False)
    b0_d = nc.declare_dram_parameter("b0", [P, G], F32, isOutput=False)
    b1_d = nc.declare_dram_parameter("b1", [P, G], F32, isOutput=False)
    readWT = nc.declare_dram_parameter("readWT", [H, M], F32, isOutput=False)
    read_b_d = nc.declare_dram_parameter("read_b", [P, M], F32, isOutput=False)
    mem_d = nc.declare_dram_parameter("memory", [M, D], F32, isOutput=False)
    clsW1T = nc.declare_dram_parameter("clsW1T", [H + D, H], F32, isOutput=False)
    cls_b1_d = nc.declare_dram_parameter("cls_b1", [P, H], F32, isOutput=False)
    clsW2T = nc.declare_dram_parameter("clsW2T", [H, 3], F32, isOutput=False)
    cls_b2_d = nc.declare_dram_parameter("cls_b2", [P, 3], F32, isOutput=False)
    out_logits = nc.declare_dram_parameter("out_logits", [BPC, 3], F32, isOutput=True)
    out_rv = nc.declare_dram_parameter("out_rv", [BPC, D], F32, isOutput=True)
    out_rw = nc.declare_dram_parameter("out_rw", [BPC, M], F32, isOutput=True)

    from contextlib import ExitStack

    with tile.TileContext(nc) as tc:
        with ExitStack() as top:
            const_pool = top.enter_context(tc.tile_pool(name="const", bufs=1))
            dram_pool = top.enter_context(tc.tile_pool(name="dram", bufs=1, space="DRAM"))
            ident = const_pool.tile([BPC, BPC], F32)
            make_identity(nc, ident)
            ident_r = const_pool.tile([BPC, BPC], F32R)
            nc.vector.tensor_copy(out=ident_r[:], in_=ident[:])
            zslab = const_pool.tile([P, 4, BPC], F32)
            nc.any.memzero(zslab[:])

            xp0_dram = dram_pool.tile([R, G], F32)
            xp1_dram = dram_pool.tile([R, G], F32)

            # ---- phase 1: Xp0 = x @ W_ih0.T + b0 ----
            with ExitStack() as ph:
                wpool = ph.enter_context(tc.tile_pool(name="w0", bufs=1))
                lpool = ph.enter_context(tc.tile_pool(name="lhsT", bufs=3))
                w_sb = wpool.tile([P, D // P, G], F32R)
                nc.sync.dma_start(
                    w_sb[:], wih0T[:].rearrange("(ko p) n -> p ko n", p=P).bitcast(F32R)
                )
                b0_bc = wpool.tile([P, G], F32)
                nc.sync.dma_start(b0_bc[:], b0_d[:])

                def lhsT_src(mo):
                    lh = lpool.tile([P, D // P, P], F32R, tag="xT")
                    nc.sync.dma_start(
                        lh[:],
                        xT[:, ts(mo, P)].rearrange("(ko p) m -> p ko m", p=P).bitcast(F32R),
                    )
                    return lh

                _projection(nc, tc, ph, R // P, D // P, lhsT_src, w_sb, b0_bc, xp0_dram)

            with ExitStack() as seq_stack:
                h0seq_pool = seq_stack.enter_context(tc.tile_pool(name="h0seq", bufs=1))
                h0seq = h0seq_pool.tile([P, 4, (s_len + 1) * BPC], F32R)
                nc.vector.tensor_copy(out=h0seq[:, :, 0:BPC], in_=zslab[:])

                # ---- phase 2: layer-0 recurrence ----
                with ExitStack() as ph:
                    wpool = ph.enter_context(tc.tile_pool(name="whh0", bufs=1))
                    w_sb = wpool.tile([P, 4, G], F32R)
                    nc.sync.dma_start(
                        w_sb[:], whh0T[:].rearrange("(ko p) n -> p ko n", p=P).bitcast(F32R)
                    )
                    c_sb = wpool.tile([BPC, H], F32)
                    nc.any.memzero(c_sb[:])
                    _lstm_recurrence(
                        nc, tc, ph, s_len, ident, ident_r, w_sb, xp0_dram,
                        hT_read=lambda t: h0seq[:, :, ts(t, BPC)],
                        hT_write=lambda t: h0seq[:, :, ts(t + 1, BPC)],
                        c_sb=c_sb,
                    )

                # ---- phase 3: Xp1 = h0_seq @ W_ih1.T + b1 ----
                with ExitStack() as ph:
                    wpool = ph.enter_context(tc.tile_pool(name="w1", bufs=1))
                    w_sb = wpool.tile([P, H // P, G], F32R)
                    nc.sync.dma_start(
                        w_sb[:], wih1T[:].rearrange("(ko p) n -> p ko n", p=P).bitcast(F32R)
                    )
                    b1_bc = wpool.tile([P, G], F32)
                    nc.sync.dma_start(b1_bc[:], b1_d[:])
                    _projection(
                        nc, tc, ph, R // P, H // P,
                        lambda mo: h0seq[:, :, BPC + mo * P : BPC + (mo + 1) * P],
                        w_sb, b1_bc, xp1_dram,
                    )

            # ---- phase 4: layer-1 recurrence ----
            with ExitStack() as ph:
                state_pool = ph.enter_context(tc.tile_pool(name="l1state", bufs=1))
                c_sb = state_pool.tile([BPC, H], F32)
                nc.any.memzero(c_sb[:])
                h1T = state_pool.tile([P, 4, 2 * BPC], F32R)
                nc.vector.tensor_copy(out=h1T[:, :, 0:BPC], in_=zslab[:])
                with ExitStack() as rec:
                    wpool = rec.enter_context(tc.tile_pool(name="whh1", bufs=1))
                    w_sb = wpool.tile([P, 4, G], F32R)
                    nc.sync.dma_start(
                        w_sb[:], whh1T[:].rearrange("(ko p) n -> p ko n", p=P).bitcast(F32R)
                    )
                    _lstm_recurrence(
                        nc, tc, rec, s_len, ident, ident_r, w_sb, xp1_dram,
                        hT_read=lambda t: h1T[:, :, ts(t % 2, BPC)],
                        hT_write=lambda t: h1T[:, :, ts((t + 1) % 2, BPC)],
                        c_sb=c_sb,
                    )
                hT_fin = h1T[:, :, ts(s_len % 2, BPC)]

                # ---- phase 5: attention read + classifier ----
                with ExitStack() as tl:
                    tpool = tl.enter_context(tc.tile_pool(name="tail", bufs=1))
                    tps = tl.enter_context(tc.tile_pool(name="tail_ps", bufs=1, space="PSUM"))

                    rw_sb = tpool.tile([BPC, M], F32)
                    rwT = tpool.tile([P, 4, BPC], F32R)
                    rv_sb = tpool.tile([BPC, D], F32)
                    rvT = tpool.tile([P, 6, BPC], F32R)
                    hid_sb = tpool.tile([BPC, H], F32)
                    hidT = tpool.tile([P, 4, BPC], F32R)
                    stat = tpool.tile([BPC, 4], F32)

                    # read_logits = h @ read_W.T + read_b
                    rwt_sb = tpool.tile([P, 4, M], F32R)
                    nc.sync.dma_start(
                        rwt_sb[:], readWT[:].rearrange("(ko p) n -> p ko n", p=P).bitcast(F32R)
                    )
                    rb_bc = tpool.tile([P, M], F32)
                    nc.sync.dma_start(rb_bc[:], read_b_d[:])
                    lg_ps = tps.tile([BPC, M], F32, tag="lg")
                    for kk in range(4):
                        nc.tensor.matmul(
                            lg_ps[:], hT_fin[:, kk, :], rwt_sb[:, kk, :],
                            start=(kk == 0), stop=(kk == 3),
                        )
                    lg_sb = tpool.tile([BPC, M], F32)
                    nc.vector.tensor_add(lg_sb[:], lg_ps[:], rb_bc[:BPC, :])

                    # softmax over M
                    nc.vector.reduce_max(stat[:, 0:1], lg_sb[:], axis=mybir.AxisListType.X)
                    nc.vector.tensor_scalar_mul(stat[:, 1:2], stat[:, 0:1], -1.0)
                    ex_sb = tpool.tile([BPC, M], F32)
                    nc.scalar.activation(ex_sb[:], lg_sb[:], AF.Exp, bias=stat[:, 1:2])
                    nc.vector.reduce_sum(stat[:, 2:3], ex_sb[:], axis=mybir.AxisListType.X)
                    nc.vector.reciprocal(stat[:, 3:4], stat[:, 2:3])
                    nc.vector.tensor_scalar_mul(rw_sb[:], ex_sb[:], stat[:, 3:4])
                    nc.sync.dma_start(out_rw[:], rw_sb[:])
                    _transpose_to_slab(nc, tps, rwT, rw_sb, 4, ident)

                    # read_vector = read_weights @ memory
                    mem_sb = tpool.tile([P, 4, D], F32R)
                    nc.sync.dma_start(
                        mem_sb[:], mem_d[:].rearrange("(ko p) n -> p ko n", p=P).bitcast(F32R)
                    )
                    rv_ps_a = tps.tile([BPC, 512], F32, tag="rva")
                    rv_ps_b = tps.tile([BPC, D - 512], F32, tag="rvb")
                    for kk in range(4):
                        nc.tensor.matmul(
                            rv_ps_a[:], rwT[:, kk, :], mem_sb[:, kk, 0:512],
                            start=(kk == 0), stop=(kk == 3),
                        )
                    for kk in range(4):
                        nc.tensor.matmul(
                            rv_ps_b[:], rwT[:, kk, :], mem_sb[:, kk, 512:D],
                            start=(kk == 0), stop=(kk == 3),
                        )
                    nc.vector.tensor_copy(out=rv_sb[:, 0:512], in_=rv_ps_a[:])
                    nc.vector.tensor_copy(out=rv_sb[:, 512:D], in_=rv_ps_b[:])
                    nc.sync.dma_start(out_rv[:], rv_sb[:])
                    _transpose_to_slab(nc, tps, rvT, rv_sb, 6, ident)

                    # hid = relu([h, rv] @ cls_W1.T + cls_b1)
                    w1_sb = tpool.tile([P, (H + D) // P, H], F32R)
                    nc.sync.dma_start(
                        w1_sb[:], clsW1T[:].rearrange("(ko p) n -> p ko n", p=P).bitcast(F32R)
                    )
                    cb1_bc = tpool.tile([P, H], F32)
                    nc.sync.dma_start(cb1_bc[:], cls_b1_d[:])
                    hid_ps = tps.tile([BPC, H], F32, tag="hid")
                    for kk in range(10):
                        lhsT = hT_fin[:, kk, :] if kk < 4 else rvT[:, kk - 4, :]
                        nc.tensor.matmul(
                            hid_ps[:], lhsT, w1_sb[:, kk, :],
                            start=(kk == 0), stop=(kk == 9),
                        )
                    nc.vector.tensor_add(hid_sb[:], hid_ps[:], cb1_bc[:BPC, :])
                    nc.scalar.activation(hid_sb[:], hid_sb[:], AF.Relu)
                    _transpose_to_slab(nc, tps, hidT, hid_sb, 4, ident)

                    # logits = hid @ cls_W2.T + cls_b2
                    w2_sb = tpool.tile([P, 4, 3], F32)
                    nc.sync.dma_start(
                        w2_sb[:], clsW2T[:].rearrange("(ko p) n -> p ko n", p=P)
                    )
                    cb2_bc = tpool.tile([P, 3], F32)
                    nc.sync.dma_start(cb2_bc[:], cls_b2_d[:])
                    lgt_ps = tps.tile([BPC, 3], F32, tag="lgt")
                    for kk in range(4):
                        nc.tensor.matmul(
                            lgt_ps[:], hidT[:, kk, :].bitcast(F32), w2_sb[:, kk, :],
                            start=(kk == 0), stop=(kk == 3),
                        )
                    lgt_sb = tpool.tile([BPC, 3], F32)
                    nc.vector.tensor_add(lgt_sb[:], lgt_ps[:], cb2_bc[:BPC, :])
                    nc.sync.dma_start(out_logits[:], lgt_sb[:])

    nc.compile()
    return nc


def _prep_shared(inputs, s_len):
    f = np.ascontiguousarray
    return {
        "wih0T": f(inputs["W_ih0"].T),
        "whh0T": f(inputs["W_hh0"].T),
        "wih1T": f(inputs["W_ih1"].T),
        "whh1T": f(inputs["W_hh1"].T),
        "b0": f(np.broadcast_to(inputs["b0"], (128, 2048))),
        "b1": f(np.broadcast_to(inputs["b1"], (128, 2048))),
        "readWT": f(inputs["read_W"].T),
        "read_b": f(np.broadcast_to(inputs["read_b"], (128, 512))),
        "memory": f(inputs["memory"]),
        "clsW1T": f(inputs["cls_W1"].T),
        "cls_b1": f(np.broadcast_to(inputs["cls_b1"], (128, 512))),
        "clsW2T": f(inputs["cls_W2"].T),
        "cls_b2": f(np.broadcast_to(inputs["cls_b2"], (128, 3))),
    }


def _run(inputs, s_len):
    global LAST_RESULTS
    inputs = {k: np.asarray(v, dtype=np.float32) for k, v in inputs.items()}
    nc = _build(s_len)
    shared = _prep_shared(inputs, s_len)
    x = inputs["legal_query"][:, :s_len]  # [B, s, D]
    in_maps = []
    for c in range(NCORES):
        xs = x[c * BPC : (c + 1) * BPC]  # [b, s, D]
        xT = np.ascontiguousarray(np.transpose(xs, (2, 1, 0)).reshape(D, s_len * BPC))
        in_maps.append({"xT": xT, **shared})
    res = run_bass_kernel_spmd(nc, in_maps, list(range(NCORES)))
    LAST_RESULTS = res
    logits = np.concatenate([res.results[c]["out_logits"] for c in range(NCORES)])
    rv = np.concatenate([res.results[c]["out_rv"] for c in range(NCORES)])
    rw = np.concatenate([res.results[c]["out_rw"] for c in range(NCORES)])
    return logits, rv, rw


def kernel(**inputs):
    return _run(inputs, S)


def kernel_truncated(inputs, s_len):
    """Development helper: run with a truncated sequence length."""
    return _run(inputs, s_len)
